# revision 10
# baseline (speedup 1.0000x reference)
"""MinGRU Trainium2 kernel.

Problem: B=8, T=4096, D=512, H=512 MinGRU:
    k = x @ Wz^T + bz;  z = sigmoid(k)
    w = x @ Wh^T + bh;  h~ = g(w),  g(w) = relu(w) + 0.5 (w>=0) | sigmoid(w) (w<0)
    h_t = (1 - z_t) * h_{t-1} + z_t * h~_t,   h_{-1} = g(h_0)
(The reference computes this recurrence in log space via cumlogsumexp; in
linear space all quantities are positive and bounded, so a direct scan with
fp32 state is numerically stable.)

Sharding: data-parallel over batch, one batch row per NeuronCore (8 cores).

Per-core device layout (everything transposed so H sits on partitions and T
on the free dim, which lets the VectorE `tensor_tensor_scan` instruction run
the recurrence along T):
    xT  (D=512, T=4096)  f32r  - host pre-transposed
    wzT/whT (D=512, H=512) f32r - host pre-transposed weights (lhsT layout)
    k^T/w^T tiles computed on PE in PSUM with float32r (full-rate fp32)
    a    = sigmoid(-k - bz)                      [ScalarE, bias/scale fused]
    s    = sigmoid(w + bh)                       [ScalarE]
    r1   = relu(w + bh)                          [ScalarE]
    g    = min(s, 0.5) + r1                      [VectorE scalar_tensor_tensor]
           (identity: sigmoid(min(v,0)) = min(sigmoid(v), 0.5))
    bneg = (a - 1) * g                           [VectorE scalar_tensor_tensor]
    h    = scan: state = a*state - bneg          [VectorE tensor_tensor_scan,
                                                  fp32 internal state]
    hT out (H=512, T=4096) -> host transposes back

The elementwise chain runs in bf16 (DVE 2x packed mode); matmuls and the
scan state stay fp32.
"""

import os

import numpy as np

import concourse.bass as bass
import concourse.mybir as mybir
import concourse.tile as tile
from concourse import bacc
from concourse.bass_utils import run_bass_kernel_spmd

# Problem constants (hardcoded per harness contract).
B, T, D, H = 8, 4096, 512, 512
P = 128          # partitions
DB = D // P      # 4 contraction blocks
HB = H // P      # 4 output h blocks
TC = 2048        # T chunk per elementwise tile
NT = T // TC     # 2
MM_N = 512       # matmul free-dim chunk
NCC = TC // MM_N # 4 matmul column chunks per tile

F32 = mybir.dt.float32
F32R = mybir.dt.float32r
BF16 = mybir.dt.bfloat16
EW = BF16        # elementwise chain dtype

# Stash of the last run's BassKernelResults (for test harness introspection).
LAST_RESULT = None


def _build_nc():
    nc = bacc.Bacc(
        "TRN2",
        target_bir_lowering=False,
        debug=False,
        enable_asserts=False,
        num_devices=B,
    )

    xT_d = nc.dram_tensor("xT", (D, T), BF16, kind="ExternalInput")
    # wT layout: (DB, P, H) so one DMA lands all four 128-row blocks side by
    # side in a single (P, DB*H) SBUF tile.
    wzT_d = nc.dram_tensor("wzT", (DB, P, H), BF16, kind="ExternalInput")
    whT_d = nc.dram_tensor("whT", (DB, P, H), BF16, kind="ExternalInput")
    # smalls columns: [0:4] -bz per h-block, [4:8] bh, [8:12] g(h_0) carries
    smalls_d = nc.dram_tensor("smalls", (P, 12), F32, kind="ExternalInput")
    hT_d = nc.dram_tensor("hT", (H, T), EW, kind="ExternalOutput")

    AF = mybir.ActivationFunctionType
    OP = mybir.AluOpType

    from contextlib import ExitStack

    with tile.TileContext(nc) as tc, ExitStack() as ctx:
        wpool = ctx.enter_context(tc.tile_pool(name="weights", bufs=1))
        xpool = ctx.enter_context(tc.tile_pool(name="xtiles", bufs=2 * DB))
        spool = ctx.enter_context(tc.tile_pool(name="work", bufs=3))
        ppool = ctx.enter_context(tc.tile_pool(name="psum", bufs=1, space="PSUM"))

        # --- Setup DMAs: first x tile, then wz (first matmul needs both),
        # then the rest; tiny smalls on the gpsimd SWDGE ring in parallel.
        smalls = wpool.tile([P, 12], F32, name="smalls")
        nc.gpsimd.dma_start(smalls[:], smalls_d.ap()[:])

        wz_sb = wpool.tile([P, DB * H], BF16, name="wz_sb")
        wh_sb = wpool.tile([P, DB * H], BF16, name="wh_sb")
        for db in range(DB):
            nc.sync.dma_start(wz_sb[:, db * H:(db + 1) * H], wzT_d.ap()[db])
        for db in range(DB):
            nc.sync.dma_start(wh_sb[:, db * H:(db + 1) * H], whT_d.ap()[db])

        def wslice(w_sb, db, hb):
            return w_sb[:, db * H + hb * P: db * H + (hb + 1) * P]

        # --- Main loop over T chunks (small first chunk primes the
        # pipeline early; small last chunk shortens the serial tail) ---
        CHUNKS = [1024, 2048, 1024]
        assert sum(CHUNKS) == T
        starts = [sum(CHUNKS[:i]) for i in range(len(CHUNKS))]

        # first chunk's x tiles (already interleaved with weight DMAs above
        # for chunk 0 -- re-issue here per chunk)
        xt_cur = None
        for ci, (ts0, clen) in enumerate(zip(starts, CHUNKS)):
            if xt_cur is None:
                xt_cur = []
                for db in range(DB):
                    x_t = xpool.tile([P, 2048], BF16, name="xt", tag="xt")
                    nc.sync.dma_start(
                        x_t[:, :clen],
                        xT_d.ap()[db * P:(db + 1) * P, ts0:ts0 + clen],
                    )
                    xt_cur.append(x_t)
            xt = xt_cur
            if ci + 1 < len(CHUNKS):
                nts0, nclen = starts[ci + 1], CHUNKS[ci + 1]
                xt_nxt = []
                for db in range(DB):
                    x_t = xpool.tile([P, 2048], BF16, name="xt", tag="xt")
                    nc.sync.dma_start(
                        x_t[:, :nclen],
                        xT_d.ap()[db * P:(db + 1) * P, nts0:nts0 + nclen],
                    )
                    xt_nxt.append(x_t)

            ncc = clen // MM_N
            for hb in range(HB):
                hs = slice(hb * P, (hb + 1) * P)

                kp = ppool.tile([P, TC], F32, name="kp", tag="kp")
                for db in range(DB):
                    for cc in range(ncc):
                        cs = slice(cc * MM_N, (cc + 1) * MM_N)
                        nc.tensor.matmul(
                            kp[:, cs],
                            wslice(wz_sb, db, hb),
                            xt[db][:, cs],
                            start=(db == 0),
                            stop=(db == DB - 1),
                        )

                a_t = spool.tile([P, TC], EW, name="a_t", tag="a")
                nc.scalar.activation(
                    a_t[:, :clen], kp[:, :clen], AF.Sigmoid,
                    bias=smalls[:, hb:hb + 1], scale=-1.0,
                )

                wp = ppool.tile([P, TC], F32, name="wp", tag="wp")
                for db in range(DB):
                    for cc in range(ncc):
                        cs = slice(cc * MM_N, (cc + 1) * MM_N)
                        nc.tensor.matmul(
                            wp[:, cs],
                            wslice(wh_sb, db, hb),
                            xt[db][:, cs],
                            start=(db == 0),
                            stop=(db == DB - 1),
                        )

                s_t = spool.tile([P, TC], EW, name="s_t", tag="s")
                nc.scalar.activation(
                    s_t[:, :clen], wp[:, :clen], AF.Sigmoid,
                    bias=smalls[:, 4 + hb:5 + hb], scale=1.0,
                )
                r_t = spool.tile([P, TC], EW, name="r_t", tag="r")
                nc.scalar.activation(
                    r_t[:, :clen], wp[:, :clen], AF.Relu,
                    bias=smalls[:, 4 + hb:5 + hb], scale=1.0,
                )

                m_t = spool.tile([P, TC], EW, name="m_t", tag="m")
                nc.vector.tensor_scalar_min(m_t[:, :clen], s_t[:, :clen], 0.5)
                g_t = spool.tile([P, TC], EW, name="g_t", tag="g")
                nc.vector.tensor_add(g_t[:, :clen], m_t[:, :clen], r_t[:, :clen])
                t_t = spool.tile([P, TC], EW, name="t_t", tag="t")
                nc.vector.tensor_scalar_sub(t_t[:, :clen], a_t[:, :clen], 1.0)
                bn_t = spool.tile([P, TC], EW, name="bn_t", tag="bn")
                nc.vector.tensor_mul(bn_t[:, :clen], t_t[:, :clen], g_t[:, :clen])

                h_t = spool.tile([P, TC], EW, name="h_t", tag="h")
                nc.vector.tensor_tensor_scan(
                    h_t[:, :clen], a_t[:, :clen], bn_t[:, :clen],
                    smalls[:, 8 + hb:9 + hb],
                    op0=OP.mult, op1=OP.subtract,
                )
                if ci + 1 < len(CHUNKS):
                    nc.vector.tensor_copy(
                        smalls[:, 8 + hb:9 + hb], h_t[:, clen - 1:clen]
                    )

                nc.sync.dma_start(hT_d.ap()[hs, ts0:ts0 + clen], h_t[:, :clen])

            if ci + 1 < len(CHUNKS):
                xt_cur = xt_nxt

    nc.compile()
    return nc


def _host_prep(x, h_0, Wz, bz, Wh, bh):
    x = np.asarray(x, dtype=np.float32)
    h_0 = np.asarray(h_0, dtype=np.float32)
    Wz = np.asarray(Wz, dtype=np.float32)
    bz = np.asarray(bz, dtype=np.float32)
    Wh = np.asarray(Wh, dtype=np.float32)
    bh = np.asarray(bh, dtype=np.float32)

    import ml_dtypes
    bf16 = ml_dtypes.bfloat16
    xT = np.ascontiguousarray(np.transpose(x, (0, 2, 1)).astype(bf16))  # (B, D, T)
    wzT = np.ascontiguousarray(Wz.T.reshape(DB, P, H).astype(bf16))  # (DB, P, H)
    whT = np.ascontiguousarray(Wh.T.reshape(DB, P, H).astype(bf16))

    # initial carry: g(h_0) = min(sigmoid(h_0), 0.5) + relu(h_0)
    sig = 1.0 / (1.0 + np.exp(-h_0.astype(np.float64)))
    h0g = (np.minimum(sig, 0.5) + np.maximum(h_0, 0.0)).astype(np.float32)

    smalls = np.zeros((B, P, 12), dtype=np.float32)
    for hb in range(HB):
        blk = slice(hb * P, (hb + 1) * P)
        smalls[:, :, hb] = -bz[blk]
        smalls[:, :, 4 + hb] = bh[blk]
        smalls[:, :, 8 + hb] = h0g[:, blk]
    smalls = np.ascontiguousarray(smalls)

    in_maps = []
    for i in range(B):
        in_maps.append({
            "xT": xT[i],
            "wzT": wzT,
            "whT": whT,
            "smalls": smalls[i],
        })
    return in_maps


def kernel(x, h_0, Wz, bz, Wh, bh):
    global LAST_RESULT
    in_maps = _host_prep(x, h_0, Wz, bz, Wh, bh)
    nc = _build_nc()
    res = run_bass_kernel_spmd(
        nc,
        in_maps,
        core_ids=list(range(B)),
        trace=bool(int(os.environ.get("MINGRU_TRACE", "0"))),
    )
    LAST_RESULT = res
    out = np.empty((B, T, H), dtype=np.float32)
    for i in range(B):
        out[i] = np.asarray(res.results[i]["hT"]).astype(np.float32).T
    return out


# revision 11
# speedup vs baseline: 1.0009x; 1.0009x over previous
"""MinGRU Trainium2 kernel.

Problem: B=8, T=4096, D=512, H=512 MinGRU:
    k = x @ Wz^T + bz;  z = sigmoid(k)
    w = x @ Wh^T + bh;  h~ = g(w),  g(w) = relu(w) + 0.5 (w>=0) | sigmoid(w) (w<0)
    h_t = (1 - z_t) * h_{t-1} + z_t * h~_t,   h_{-1} = g(h_0)
(The reference computes this recurrence in log space via cumlogsumexp; in
linear space all quantities are positive and bounded, so a direct scan with
fp32 state is numerically stable.)

Sharding: data-parallel over batch, one batch row per NeuronCore (8 cores).

Per-core device layout (everything transposed so H sits on partitions and T
on the free dim, which lets the VectorE `tensor_tensor_scan` instruction run
the recurrence along T):
    xT  (D=512, T=4096)  f32r  - host pre-transposed
    wzT/whT (D=512, H=512) f32r - host pre-transposed weights (lhsT layout)
    k^T/w^T tiles computed on PE in PSUM with float32r (full-rate fp32)
    a    = sigmoid(-k - bz)                      [ScalarE, bias/scale fused]
    s    = sigmoid(w + bh)                       [ScalarE]
    r1   = relu(w + bh)                          [ScalarE]
    g    = min(s, 0.5) + r1                      [VectorE scalar_tensor_tensor]
           (identity: sigmoid(min(v,0)) = min(sigmoid(v), 0.5))
    bneg = (a - 1) * g                           [VectorE scalar_tensor_tensor]
    h    = scan: state = a*state - bneg          [VectorE tensor_tensor_scan,
                                                  fp32 internal state]
    hT out (H=512, T=4096) -> host transposes back

The elementwise chain runs in bf16 (DVE 2x packed mode); matmuls and the
scan state stay fp32.
"""

import os

import numpy as np

import concourse.bass as bass
import concourse.mybir as mybir
import concourse.tile as tile
from concourse import bacc
from concourse.bass_utils import run_bass_kernel_spmd

# Problem constants (hardcoded per harness contract).
B, T, D, H = 8, 4096, 512, 512
P = 128          # partitions
DB = D // P      # 4 contraction blocks
HB = H // P      # 4 output h blocks
TC = 2048        # T chunk per elementwise tile
NT = T // TC     # 2
MM_N = 512       # matmul free-dim chunk
NCC = TC // MM_N # 4 matmul column chunks per tile

F32 = mybir.dt.float32
F32R = mybir.dt.float32r
BF16 = mybir.dt.bfloat16
EW = BF16        # elementwise chain dtype

# Stash of the last run's BassKernelResults (for test harness introspection).
LAST_RESULT = None


def _build_nc():
    nc = bacc.Bacc(
        "TRN2",
        target_bir_lowering=False,
        debug=False,
        enable_asserts=False,
        num_devices=B,
    )

    xT_d = nc.dram_tensor("xT", (D, T), BF16, kind="ExternalInput")
    # wT layout: (DB, P, H) so one DMA lands all four 128-row blocks side by
    # side in a single (P, DB*H) SBUF tile.
    wzT_d = nc.dram_tensor("wzT", (DB, P, H), BF16, kind="ExternalInput")
    whT_d = nc.dram_tensor("whT", (DB, P, H), BF16, kind="ExternalInput")
    # smalls columns: [0:4] -bz per h-block, [4:8] bh, [8:12] g(h_0) carries
    smalls_d = nc.dram_tensor("smalls", (P, 12), F32, kind="ExternalInput")
    hT_d = nc.dram_tensor("hT", (H, T), EW, kind="ExternalOutput")

    AF = mybir.ActivationFunctionType
    OP = mybir.AluOpType

    from contextlib import ExitStack

    with tile.TileContext(nc) as tc, ExitStack() as ctx:
        wpool = ctx.enter_context(tc.tile_pool(name="weights", bufs=1))
        xpool = ctx.enter_context(tc.tile_pool(name="xtiles", bufs=2 * DB))
        spool = ctx.enter_context(tc.tile_pool(name="work", bufs=3))
        ppool = ctx.enter_context(tc.tile_pool(name="psum", bufs=1, space="PSUM"))

        # --- Setup DMAs: first x tile, then wz (first matmul needs both),
        # then the rest; tiny smalls on the gpsimd SWDGE ring in parallel.
        smalls = wpool.tile([P, 12], F32, name="smalls")
        nc.gpsimd.dma_start(smalls[:], smalls_d.ap()[:])

        wz_sb = wpool.tile([P, DB * H], BF16, name="wz_sb")
        wh_sb = wpool.tile([P, DB * H], BF16, name="wh_sb")
        for db in range(DB):
            nc.sync.dma_start(wz_sb[:, db * H:(db + 1) * H], wzT_d.ap()[db])
        for db in range(DB):
            nc.sync.dma_start(wh_sb[:, db * H:(db + 1) * H], whT_d.ap()[db])

        def wslice(w_sb, db, hb):
            return w_sb[:, db * H + hb * P: db * H + (hb + 1) * P]

        # --- Main loop over T chunks (small first chunk primes the
        # pipeline early; small last chunk shortens the serial tail) ---
        CHUNKS = [2048, 2048]
        assert sum(CHUNKS) == T
        starts = [sum(CHUNKS[:i]) for i in range(len(CHUNKS))]

        # first chunk's x tiles (already interleaved with weight DMAs above
        # for chunk 0 -- re-issue here per chunk)
        xt_cur = None
        for ci, (ts0, clen) in enumerate(zip(starts, CHUNKS)):
            if xt_cur is None:
                xt_cur = []
                for db in range(DB):
                    x_t = xpool.tile([P, 2048], BF16, name="xt", tag="xt")
                    nc.sync.dma_start(
                        x_t[:, :clen],
                        xT_d.ap()[db * P:(db + 1) * P, ts0:ts0 + clen],
                    )
                    xt_cur.append(x_t)
            xt = xt_cur
            if ci + 1 < len(CHUNKS):
                nts0, nclen = starts[ci + 1], CHUNKS[ci + 1]
                xt_nxt = []
                for db in range(DB):
                    x_t = xpool.tile([P, 2048], BF16, name="xt", tag="xt")
                    nc.sync.dma_start(
                        x_t[:, :nclen],
                        xT_d.ap()[db * P:(db + 1) * P, nts0:nts0 + nclen],
                    )
                    xt_nxt.append(x_t)

            ncc = clen // MM_N
            for hb in range(HB):
                hs = slice(hb * P, (hb + 1) * P)

                kp = ppool.tile([P, TC], F32, name="kp", tag="kp")
                for db in range(DB):
                    for cc in range(ncc):
                        cs = slice(cc * MM_N, (cc + 1) * MM_N)
                        nc.tensor.matmul(
                            kp[:, cs],
                            wslice(wz_sb, db, hb),
                            xt[db][:, cs],
                            start=(db == 0),
                            stop=(db == DB - 1),
                        )

                a_t = spool.tile([P, TC], EW, name="a_t", tag="a")
                nc.scalar.activation(
                    a_t[:, :clen], kp[:, :clen], AF.Sigmoid,
                    bias=smalls[:, hb:hb + 1], scale=-1.0,
                )

                wp = ppool.tile([P, TC], F32, name="wp", tag="wp")
                for db in range(DB):
                    for cc in range(ncc):
                        cs = slice(cc * MM_N, (cc + 1) * MM_N)
                        nc.tensor.matmul(
                            wp[:, cs],
                            wslice(wh_sb, db, hb),
                            xt[db][:, cs],
                            start=(db == 0),
                            stop=(db == DB - 1),
                        )

                s_t = spool.tile([P, TC], EW, name="s_t", tag="s")
                nc.scalar.activation(
                    s_t[:, :clen], wp[:, :clen], AF.Sigmoid,
                    bias=smalls[:, 4 + hb:5 + hb], scale=1.0,
                )
                r_t = spool.tile([P, TC], EW, name="r_t", tag="r")
                nc.scalar.activation(
                    r_t[:, :clen], wp[:, :clen], AF.Relu,
                    bias=smalls[:, 4 + hb:5 + hb], scale=1.0,
                )

                m_t = spool.tile([P, TC], EW, name="m_t", tag="m")
                nc.vector.tensor_scalar_min(m_t[:, :clen], s_t[:, :clen], 0.5)
                g_t = spool.tile([P, TC], EW, name="g_t", tag="g")
                nc.vector.tensor_add(g_t[:, :clen], m_t[:, :clen], r_t[:, :clen])
                t_t = spool.tile([P, TC], EW, name="t_t", tag="t")
                nc.vector.tensor_scalar_sub(t_t[:, :clen], a_t[:, :clen], 1.0)
                bn_t = spool.tile([P, TC], EW, name="bn_t", tag="bn")
                nc.vector.tensor_mul(bn_t[:, :clen], t_t[:, :clen], g_t[:, :clen])

                h_t = spool.tile([P, TC], EW, name="h_t", tag="h")
                last_tile = (ci == len(CHUNKS) - 1) and (hb == HB - 1)
                if last_tile:
                    half = clen // 2
                    nc.vector.tensor_tensor_scan(
                        h_t[:, :half], a_t[:, :half], bn_t[:, :half],
                        smalls[:, 8 + hb:9 + hb],
                        op0=OP.mult, op1=OP.subtract,
                    )
                    nc.sync.dma_start(
                        hT_d.ap()[hs, ts0:ts0 + half], h_t[:, :half]
                    )
                    nc.vector.tensor_tensor_scan(
                        h_t[:, half:clen], a_t[:, half:clen],
                        bn_t[:, half:clen], h_t[:, half - 1:half],
                        op0=OP.mult, op1=OP.subtract,
                    )
                    nc.sync.dma_start(
                        hT_d.ap()[hs, ts0 + half:ts0 + clen], h_t[:, half:clen]
                    )
                else:
                    nc.vector.tensor_tensor_scan(
                        h_t[:, :clen], a_t[:, :clen], bn_t[:, :clen],
                        smalls[:, 8 + hb:9 + hb],
                        op0=OP.mult, op1=OP.subtract,
                    )
                    if ci + 1 < len(CHUNKS):
                        nc.vector.tensor_copy(
                            smalls[:, 8 + hb:9 + hb], h_t[:, clen - 1:clen]
                        )
                    nc.sync.dma_start(
                        hT_d.ap()[hs, ts0:ts0 + clen], h_t[:, :clen]
                    )

            if ci + 1 < len(CHUNKS):
                xt_cur = xt_nxt

    nc.compile()
    return nc


def _host_prep(x, h_0, Wz, bz, Wh, bh):
    x = np.asarray(x, dtype=np.float32)
    h_0 = np.asarray(h_0, dtype=np.float32)
    Wz = np.asarray(Wz, dtype=np.float32)
    bz = np.asarray(bz, dtype=np.float32)
    Wh = np.asarray(Wh, dtype=np.float32)
    bh = np.asarray(bh, dtype=np.float32)

    import ml_dtypes
    bf16 = ml_dtypes.bfloat16
    xT = np.ascontiguousarray(np.transpose(x, (0, 2, 1)).astype(bf16))  # (B, D, T)
    wzT = np.ascontiguousarray(Wz.T.reshape(DB, P, H).astype(bf16))  # (DB, P, H)
    whT = np.ascontiguousarray(Wh.T.reshape(DB, P, H).astype(bf16))

    # initial carry: g(h_0) = min(sigmoid(h_0), 0.5) + relu(h_0)
    sig = 1.0 / (1.0 + np.exp(-h_0.astype(np.float64)))
    h0g = (np.minimum(sig, 0.5) + np.maximum(h_0, 0.0)).astype(np.float32)

    smalls = np.zeros((B, P, 12), dtype=np.float32)
    for hb in range(HB):
        blk = slice(hb * P, (hb + 1) * P)
        smalls[:, :, hb] = -bz[blk]
        smalls[:, :, 4 + hb] = bh[blk]
        smalls[:, :, 8 + hb] = h0g[:, blk]
    smalls = np.ascontiguousarray(smalls)

    in_maps = []
    for i in range(B):
        in_maps.append({
            "xT": xT[i],
            "wzT": wzT,
            "whT": whT,
            "smalls": smalls[i],
        })
    return in_maps


def kernel(x, h_0, Wz, bz, Wh, bh):
    global LAST_RESULT
    in_maps = _host_prep(x, h_0, Wz, bz, Wh, bh)
    nc = _build_nc()
    res = run_bass_kernel_spmd(
        nc,
        in_maps,
        core_ids=list(range(B)),
        trace=bool(int(os.environ.get("MINGRU_TRACE", "0"))),
    )
    LAST_RESULT = res
    out = np.empty((B, T, H), dtype=np.float32)
    for i in range(B):
        out[i] = np.asarray(res.results[i]["hT"]).astype(np.float32).T
    return out


# revision 12
# speedup vs baseline: 1.0033x; 1.0024x over previous
"""MinGRU Trainium2 kernel.

Problem: B=8, T=4096, D=512, H=512 MinGRU:
    k = x @ Wz^T + bz;  z = sigmoid(k)
    w = x @ Wh^T + bh;  h~ = g(w),  g(w) = relu(w) + 0.5 (w>=0) | sigmoid(w) (w<0)
    h_t = (1 - z_t) * h_{t-1} + z_t * h~_t,   h_{-1} = g(h_0)
(The reference computes this recurrence in log space via cumlogsumexp; in
linear space all quantities are positive and bounded, so a direct scan with
fp32 state is numerically stable.)

Sharding: data-parallel over batch, one batch row per NeuronCore (8 cores).

Per-core device layout (everything transposed so H sits on partitions and T
on the free dim, which lets the VectorE `tensor_tensor_scan` instruction run
the recurrence along T):
    xT  (D=512, T=4096)  f32r  - host pre-transposed
    wzT/whT (D=512, H=512) f32r - host pre-transposed weights (lhsT layout)
    k^T/w^T tiles computed on PE in PSUM with float32r (full-rate fp32)
    a    = sigmoid(-k - bz)                      [ScalarE, bias/scale fused]
    s    = sigmoid(w + bh)                       [ScalarE]
    r1   = relu(w + bh)                          [ScalarE]
    g    = min(s, 0.5) + r1                      [VectorE scalar_tensor_tensor]
           (identity: sigmoid(min(v,0)) = min(sigmoid(v), 0.5))
    bneg = (a - 1) * g                           [VectorE scalar_tensor_tensor]
    h    = scan: state = a*state - bneg          [VectorE tensor_tensor_scan,
                                                  fp32 internal state]
    hT out (H=512, T=4096) -> host transposes back

The elementwise chain runs in bf16 (DVE 2x packed mode); matmuls and the
scan state stay fp32.
"""

import os

import numpy as np

import concourse.bass as bass
import concourse.mybir as mybir
import concourse.tile as tile
from concourse import bacc
from concourse.bass_utils import run_bass_kernel_spmd

# Problem constants (hardcoded per harness contract).
B, T, D, H = 8, 4096, 512, 512
P = 128          # partitions
DB = D // P      # 4 contraction blocks
HB = H // P      # 4 output h blocks
TC = 2048        # T chunk per elementwise tile
NT = T // TC     # 2
MM_N = 512       # matmul free-dim chunk
NCC = TC // MM_N # 4 matmul column chunks per tile

F32 = mybir.dt.float32
F32R = mybir.dt.float32r
BF16 = mybir.dt.bfloat16
EW = BF16        # elementwise chain dtype

# Stash of the last run's BassKernelResults (for test harness introspection).
LAST_RESULT = None


def _build_nc():
    nc = bacc.Bacc(
        "TRN2",
        target_bir_lowering=False,
        debug=False,
        enable_asserts=False,
        num_devices=B,
    )

    xT_d = nc.dram_tensor("xT", (D, T), BF16, kind="ExternalInput")
    # wT layout: (DB, P, H) so one DMA lands all four 128-row blocks side by
    # side in a single (P, DB*H) SBUF tile.
    wzT_d = nc.dram_tensor("wzT", (DB, P, H), BF16, kind="ExternalInput")
    whT_d = nc.dram_tensor("whT", (DB, P, H), BF16, kind="ExternalInput")
    # smalls columns: [0:4] -bz per h-block, [4:8] bh, [8:12] g(h_0) carries
    smalls_d = nc.dram_tensor("smalls", (P, 12), F32, kind="ExternalInput")
    hT_d = nc.dram_tensor("hT", (H, T), EW, kind="ExternalOutput")

    AF = mybir.ActivationFunctionType
    OP = mybir.AluOpType

    from contextlib import ExitStack

    with tile.TileContext(nc) as tc, ExitStack() as ctx:
        wpool = ctx.enter_context(tc.tile_pool(name="weights", bufs=1))
        xpool = ctx.enter_context(tc.tile_pool(name="xtiles", bufs=2 * DB))
        spool = ctx.enter_context(tc.tile_pool(name="work", bufs=3))
        ppool = ctx.enter_context(tc.tile_pool(name="psum", bufs=1, space="PSUM"))

        # --- Setup DMAs: first x tile, then wz (first matmul needs both),
        # then the rest; tiny smalls on the gpsimd SWDGE ring in parallel.
        smalls = wpool.tile([P, 12], F32, name="smalls")
        nc.gpsimd.dma_start(smalls[:], smalls_d.ap()[:])

        wz_sb = wpool.tile([P, DB * H], BF16, name="wz_sb")
        wh_sb = wpool.tile([P, DB * H], BF16, name="wh_sb")
        xt0 = []
        for db in range(DB):
            x_t = xpool.tile([P, 2048], BF16, name="xt", tag="xt")
            nc.sync.dma_start(x_t[:], xT_d.ap()[db * P:(db + 1) * P, 0:2048])
            xt0.append(x_t)
        for db in range(DB):
            nc.sync.dma_start(wz_sb[:, db * H:(db + 1) * H], wzT_d.ap()[db])
        for db in range(DB):
            nc.sync.dma_start(wh_sb[:, db * H:(db + 1) * H], whT_d.ap()[db])

        def wslice(w_sb, db, hb):
            return w_sb[:, db * H + hb * P: db * H + (hb + 1) * P]

        # --- Main loop over T chunks (small first chunk primes the
        # pipeline early; small last chunk shortens the serial tail) ---
        CHUNKS = [2048, 2048]
        assert sum(CHUNKS) == T
        starts = [sum(CHUNKS[:i]) for i in range(len(CHUNKS))]

        # first chunk's x tiles (already interleaved with weight DMAs above
        # for chunk 0 -- re-issue here per chunk)
        xt_cur = xt0
        for ci, (ts0, clen) in enumerate(zip(starts, CHUNKS)):
            xt = xt_cur
            if ci + 1 < len(CHUNKS):
                nts0, nclen = starts[ci + 1], CHUNKS[ci + 1]
                xt_nxt = []
                for db in range(DB):
                    x_t = xpool.tile([P, 2048], BF16, name="xt", tag="xt")
                    nc.sync.dma_start(
                        x_t[:, :nclen],
                        xT_d.ap()[db * P:(db + 1) * P, nts0:nts0 + nclen],
                    )
                    xt_nxt.append(x_t)

            ncc = clen // MM_N
            for hb in range(HB):
                hs = slice(hb * P, (hb + 1) * P)

                kp = ppool.tile([P, TC], F32, name="kp", tag="kp")
                for db in range(DB):
                    for cc in range(ncc):
                        cs = slice(cc * MM_N, (cc + 1) * MM_N)
                        nc.tensor.matmul(
                            kp[:, cs],
                            wslice(wz_sb, db, hb),
                            xt[db][:, cs],
                            start=(db == 0),
                            stop=(db == DB - 1),
                        )

                a_t = spool.tile([P, TC], EW, name="a_t", tag="a")
                nc.scalar.activation(
                    a_t[:, :clen], kp[:, :clen], AF.Sigmoid,
                    bias=smalls[:, hb:hb + 1], scale=-1.0,
                )

                wp = ppool.tile([P, TC], F32, name="wp", tag="wp")
                for db in range(DB):
                    for cc in range(ncc):
                        cs = slice(cc * MM_N, (cc + 1) * MM_N)
                        nc.tensor.matmul(
                            wp[:, cs],
                            wslice(wh_sb, db, hb),
                            xt[db][:, cs],
                            start=(db == 0),
                            stop=(db == DB - 1),
                        )

                s_t = spool.tile([P, TC], EW, name="s_t", tag="s")
                nc.scalar.activation(
                    s_t[:, :clen], wp[:, :clen], AF.Sigmoid,
                    bias=smalls[:, 4 + hb:5 + hb], scale=1.0,
                )
                r_t = spool.tile([P, TC], EW, name="r_t", tag="r")
                nc.scalar.activation(
                    r_t[:, :clen], wp[:, :clen], AF.Relu,
                    bias=smalls[:, 4 + hb:5 + hb], scale=1.0,
                )

                m_t = spool.tile([P, TC], EW, name="m_t", tag="m")
                nc.vector.tensor_scalar_min(m_t[:, :clen], s_t[:, :clen], 0.5)
                g_t = spool.tile([P, TC], EW, name="g_t", tag="g")
                nc.vector.tensor_add(g_t[:, :clen], m_t[:, :clen], r_t[:, :clen])
                t_t = spool.tile([P, TC], EW, name="t_t", tag="t")
                nc.vector.tensor_scalar_sub(t_t[:, :clen], a_t[:, :clen], 1.0)
                bn_t = spool.tile([P, TC], EW, name="bn_t", tag="bn")
                nc.vector.tensor_mul(bn_t[:, :clen], t_t[:, :clen], g_t[:, :clen])

                h_t = spool.tile([P, TC], EW, name="h_t", tag="h")
                last_tile = (ci == len(CHUNKS) - 1) and (hb == HB - 1)
                if last_tile:
                    half = clen // 2
                    nc.vector.tensor_tensor_scan(
                        h_t[:, :half], a_t[:, :half], bn_t[:, :half],
                        smalls[:, 8 + hb:9 + hb],
                        op0=OP.mult, op1=OP.subtract,
                    )
                    nc.sync.dma_start(
                        hT_d.ap()[hs, ts0:ts0 + half], h_t[:, :half]
                    )
                    nc.vector.tensor_tensor_scan(
                        h_t[:, half:clen], a_t[:, half:clen],
                        bn_t[:, half:clen], h_t[:, half - 1:half],
                        op0=OP.mult, op1=OP.subtract,
                    )
                    nc.sync.dma_start(
                        hT_d.ap()[hs, ts0 + half:ts0 + clen], h_t[:, half:clen]
                    )
                else:
                    nc.vector.tensor_tensor_scan(
                        h_t[:, :clen], a_t[:, :clen], bn_t[:, :clen],
                        smalls[:, 8 + hb:9 + hb],
                        op0=OP.mult, op1=OP.subtract,
                    )
                    if ci + 1 < len(CHUNKS):
                        nc.vector.tensor_copy(
                            smalls[:, 8 + hb:9 + hb], h_t[:, clen - 1:clen]
                        )
                    nc.sync.dma_start(
                        hT_d.ap()[hs, ts0:ts0 + clen], h_t[:, :clen]
                    )

            if ci + 1 < len(CHUNKS):
                xt_cur = xt_nxt

    nc.compile()
    return nc


def _host_prep(x, h_0, Wz, bz, Wh, bh):
    x = np.asarray(x, dtype=np.float32)
    h_0 = np.asarray(h_0, dtype=np.float32)
    Wz = np.asarray(Wz, dtype=np.float32)
    bz = np.asarray(bz, dtype=np.float32)
    Wh = np.asarray(Wh, dtype=np.float32)
    bh = np.asarray(bh, dtype=np.float32)

    import ml_dtypes
    bf16 = ml_dtypes.bfloat16
    xT = np.ascontiguousarray(np.transpose(x, (0, 2, 1)).astype(bf16))  # (B, D, T)
    wzT = np.ascontiguousarray(Wz.T.reshape(DB, P, H).astype(bf16))  # (DB, P, H)
    whT = np.ascontiguousarray(Wh.T.reshape(DB, P, H).astype(bf16))

    # initial carry: g(h_0) = min(sigmoid(h_0), 0.5) + relu(h_0)
    sig = 1.0 / (1.0 + np.exp(-h_0.astype(np.float64)))
    h0g = (np.minimum(sig, 0.5) + np.maximum(h_0, 0.0)).astype(np.float32)

    smalls = np.zeros((B, P, 12), dtype=np.float32)
    for hb in range(HB):
        blk = slice(hb * P, (hb + 1) * P)
        smalls[:, :, hb] = -bz[blk]
        smalls[:, :, 4 + hb] = bh[blk]
        smalls[:, :, 8 + hb] = h0g[:, blk]
    smalls = np.ascontiguousarray(smalls)

    in_maps = []
    for i in range(B):
        in_maps.append({
            "xT": xT[i],
            "wzT": wzT,
            "whT": whT,
            "smalls": smalls[i],
        })
    return in_maps


def kernel(x, h_0, Wz, bz, Wh, bh):
    global LAST_RESULT
    in_maps = _host_prep(x, h_0, Wz, bz, Wh, bh)
    nc = _build_nc()
    res = run_bass_kernel_spmd(
        nc,
        in_maps,
        core_ids=list(range(B)),
        trace=bool(int(os.environ.get("MINGRU_TRACE", "0"))),
    )
    LAST_RESULT = res
    out = np.empty((B, T, H), dtype=np.float32)
    for i in range(B):
        out[i] = np.asarray(res.results[i]["hT"]).astype(np.float32).T
    return out


# revision 13
# speedup vs baseline: 1.0170x; 1.0137x over previous
"""MinGRU Trainium2 kernel.

Problem: B=8, T=4096, D=512, H=512 MinGRU:
    k = x @ Wz^T + bz;  z = sigmoid(k)
    w = x @ Wh^T + bh;  h~ = g(w),  g(w) = relu(w) + 0.5 (w>=0) | sigmoid(w) (w<0)
    h_t = (1 - z_t) * h_{t-1} + z_t * h~_t,   h_{-1} = g(h_0)
(The reference computes this recurrence in log space via cumlogsumexp; in
linear space all quantities are positive and bounded, so a direct scan with
fp32 state is numerically stable.)

Sharding: data-parallel over batch, one batch row per NeuronCore (8 cores).

Per-core device layout (everything transposed so H sits on partitions and T
on the free dim, which lets the VectorE `tensor_tensor_scan` instruction run
the recurrence along T):
    xT  (D=512, T=4096)  f32r  - host pre-transposed
    wzT/whT (D=512, H=512) f32r - host pre-transposed weights (lhsT layout)
    k^T/w^T tiles computed on PE in PSUM with float32r (full-rate fp32)
    a    = sigmoid(-k - bz)                      [ScalarE, bias/scale fused]
    s    = sigmoid(w + bh)                       [ScalarE]
    r1   = relu(w + bh)                          [ScalarE]
    g    = min(s, 0.5) + r1                      [VectorE scalar_tensor_tensor]
           (identity: sigmoid(min(v,0)) = min(sigmoid(v), 0.5))
    bneg = (a - 1) * g                           [VectorE scalar_tensor_tensor]
    h    = scan: state = a*state - bneg          [VectorE tensor_tensor_scan,
                                                  fp32 internal state]
    hT out (H=512, T=4096) -> host transposes back

The elementwise chain runs in bf16 (DVE 2x packed mode); matmuls and the
scan state stay fp32.
"""

import os

import numpy as np

import concourse.bass as bass
import concourse.mybir as mybir
import concourse.tile as tile
from concourse import bacc
from concourse.bass_utils import run_bass_kernel_spmd

# Problem constants (hardcoded per harness contract).
B, T, D, H = 8, 4096, 512, 512
P = 128          # partitions
DB = D // P      # 4 contraction blocks
HB = H // P      # 4 output h blocks
TC = 2048        # T chunk per elementwise tile
NT = T // TC     # 2
MM_N = 512       # matmul free-dim chunk
NCC = TC // MM_N # 4 matmul column chunks per tile

F32 = mybir.dt.float32
F32R = mybir.dt.float32r
BF16 = mybir.dt.bfloat16
EW = BF16        # elementwise chain dtype

# Stash of the last run's BassKernelResults (for test harness introspection).
LAST_RESULT = None


def _build_nc():
    nc = bacc.Bacc(
        "TRN2",
        target_bir_lowering=False,
        debug=False,
        enable_asserts=False,
        num_devices=B,
    )

    xT_d = nc.dram_tensor("xT", (D, T), BF16, kind="ExternalInput")
    # wT layout: (DB, P, H) so one DMA lands all four 128-row blocks side by
    # side in a single (P, DB*H) SBUF tile.
    wzT_d = nc.dram_tensor("wzT", (DB, P, H), BF16, kind="ExternalInput")
    whT_d = nc.dram_tensor("whT", (DB, P, H), BF16, kind="ExternalInput")
    # smalls columns: [0:4] -bz per h-block, [4:8] bh, [8:12] g(h_0) carries
    smalls_d = nc.dram_tensor("smalls", (P, 16), F32, kind="ExternalInput")
    hT_d = nc.dram_tensor("hT", (H, T), EW, kind="ExternalOutput")

    AF = mybir.ActivationFunctionType
    OP = mybir.AluOpType

    from contextlib import ExitStack

    with tile.TileContext(nc) as tc, ExitStack() as ctx:
        wpool = ctx.enter_context(tc.tile_pool(name="weights", bufs=1))
        xpool = ctx.enter_context(tc.tile_pool(name="xtiles", bufs=2 * DB))
        spool = ctx.enter_context(tc.tile_pool(name="work", bufs=3))
        ppool = ctx.enter_context(tc.tile_pool(name="psum", bufs=1, space="PSUM"))

        # --- Setup DMAs: first x tile, then wz (first matmul needs both),
        # then the rest; tiny smalls on the gpsimd SWDGE ring in parallel.
        smalls = wpool.tile([P, 16], F32, name="smalls")
        nc.gpsimd.dma_start(smalls[:], smalls_d.ap()[:])

        wz_sb = wpool.tile([P, DB * H], BF16, name="wz_sb")
        wh_sb = wpool.tile([P, DB * H], BF16, name="wh_sb")
        xt0 = []
        for db in range(DB):
            x_t = xpool.tile([P, 2048], BF16, name="xt", tag="xt")
            nc.sync.dma_start(x_t[:], xT_d.ap()[db * P:(db + 1) * P, 0:2048])
            xt0.append(x_t)
        for db in range(DB):
            nc.sync.dma_start(wz_sb[:, db * H:(db + 1) * H], wzT_d.ap()[db])
        for db in range(DB):
            nc.sync.dma_start(wh_sb[:, db * H:(db + 1) * H], whT_d.ap()[db])

        def wslice(w_sb, db, hb):
            return w_sb[:, db * H + hb * P: db * H + (hb + 1) * P]

        # --- Main loop over T chunks (small first chunk primes the
        # pipeline early; small last chunk shortens the serial tail) ---
        CHUNKS = [2048, 2048]
        assert sum(CHUNKS) == T
        starts = [sum(CHUNKS[:i]) for i in range(len(CHUNKS))]

        # first chunk's x tiles (already interleaved with weight DMAs above
        # for chunk 0 -- re-issue here per chunk)
        xt_cur = xt0
        for ci, (ts0, clen) in enumerate(zip(starts, CHUNKS)):
            xt = xt_cur
            if ci + 1 < len(CHUNKS):
                nts0, nclen = starts[ci + 1], CHUNKS[ci + 1]
                xt_nxt = []
                for db in range(DB):
                    x_t = xpool.tile([P, 2048], BF16, name="xt", tag="xt")
                    nc.sync.dma_start(
                        x_t[:, :nclen],
                        xT_d.ap()[db * P:(db + 1) * P, nts0:nts0 + nclen],
                    )
                    xt_nxt.append(x_t)

            ncc = clen // MM_N
            for hb in range(HB):
                hs = slice(hb * P, (hb + 1) * P)

                kp = ppool.tile([P, TC], F32, name="kp", tag="kp")
                for db in range(DB):
                    for cc in range(ncc):
                        cs = slice(cc * MM_N, (cc + 1) * MM_N)
                        nc.tensor.matmul(
                            kp[:, cs],
                            wslice(wz_sb, db, hb),
                            xt[db][:, cs],
                            start=(db == 0),
                            stop=(db == DB - 1),
                        )

                a_t = spool.tile([P, TC], EW, name="a_t", tag="a")
                nc.scalar.activation(
                    a_t[:, :clen], kp[:, :clen], AF.Sigmoid,
                    bias=smalls[:, hb:hb + 1], scale=-1.0,
                )
                z_t = spool.tile([P, TC], EW, name="z_t", tag="z")
                nc.scalar.activation(
                    z_t[:, :clen], kp[:, :clen], AF.Sigmoid,
                    bias=smalls[:, 12 + hb:13 + hb], scale=1.0,
                )

                wp = ppool.tile([P, TC], F32, name="wp", tag="wp")
                for db in range(DB):
                    for cc in range(ncc):
                        cs = slice(cc * MM_N, (cc + 1) * MM_N)
                        nc.tensor.matmul(
                            wp[:, cs],
                            wslice(wh_sb, db, hb),
                            xt[db][:, cs],
                            start=(db == 0),
                            stop=(db == DB - 1),
                        )

                s_t = spool.tile([P, TC], EW, name="s_t", tag="s")
                nc.scalar.activation(
                    s_t[:, :clen], wp[:, :clen], AF.Sigmoid,
                    bias=smalls[:, 4 + hb:5 + hb], scale=1.0,
                )
                r_t = spool.tile([P, TC], EW, name="r_t", tag="r")
                nc.scalar.activation(
                    r_t[:, :clen], wp[:, :clen], AF.Relu,
                    bias=smalls[:, 4 + hb:5 + hb], scale=1.0,
                )

                m_t = spool.tile([P, TC], EW, name="m_t", tag="m")
                nc.vector.tensor_scalar_min(m_t[:, :clen], s_t[:, :clen], 0.5)
                g_t = spool.tile([P, TC], EW, name="g_t", tag="g")
                nc.vector.tensor_add(g_t[:, :clen], m_t[:, :clen], r_t[:, :clen])
                bn_t = spool.tile([P, TC], EW, name="bn_t", tag="bn")
                nc.vector.tensor_mul(bn_t[:, :clen], z_t[:, :clen], g_t[:, :clen])

                h_t = spool.tile([P, TC], EW, name="h_t", tag="h")
                last_tile = (ci == len(CHUNKS) - 1) and (hb == HB - 1)
                if last_tile:
                    half = clen // 2
                    nc.vector.tensor_tensor_scan(
                        h_t[:, :half], a_t[:, :half], bn_t[:, :half],
                        smalls[:, 8 + hb:9 + hb],
                        op0=OP.mult, op1=OP.add,
                    )
                    nc.sync.dma_start(
                        hT_d.ap()[hs, ts0:ts0 + half], h_t[:, :half]
                    )
                    nc.vector.tensor_tensor_scan(
                        h_t[:, half:clen], a_t[:, half:clen],
                        bn_t[:, half:clen], h_t[:, half - 1:half],
                        op0=OP.mult, op1=OP.add,
                    )
                    nc.sync.dma_start(
                        hT_d.ap()[hs, ts0 + half:ts0 + clen], h_t[:, half:clen]
                    )
                else:
                    nc.vector.tensor_tensor_scan(
                        h_t[:, :clen], a_t[:, :clen], bn_t[:, :clen],
                        smalls[:, 8 + hb:9 + hb],
                        op0=OP.mult, op1=OP.add,
                    )
                    if ci + 1 < len(CHUNKS):
                        nc.vector.tensor_copy(
                            smalls[:, 8 + hb:9 + hb], h_t[:, clen - 1:clen]
                        )
                    nc.sync.dma_start(
                        hT_d.ap()[hs, ts0:ts0 + clen], h_t[:, :clen]
                    )

            if ci + 1 < len(CHUNKS):
                xt_cur = xt_nxt

    nc.compile()
    return nc


def _host_prep(x, h_0, Wz, bz, Wh, bh):
    x = np.asarray(x, dtype=np.float32)
    h_0 = np.asarray(h_0, dtype=np.float32)
    Wz = np.asarray(Wz, dtype=np.float32)
    bz = np.asarray(bz, dtype=np.float32)
    Wh = np.asarray(Wh, dtype=np.float32)
    bh = np.asarray(bh, dtype=np.float32)

    import ml_dtypes
    bf16 = ml_dtypes.bfloat16
    xT = np.ascontiguousarray(np.transpose(x, (0, 2, 1)).astype(bf16))  # (B, D, T)
    wzT = np.ascontiguousarray(Wz.T.reshape(DB, P, H).astype(bf16))  # (DB, P, H)
    whT = np.ascontiguousarray(Wh.T.reshape(DB, P, H).astype(bf16))

    # initial carry: g(h_0) = min(sigmoid(h_0), 0.5) + relu(h_0)
    sig = 1.0 / (1.0 + np.exp(-h_0.astype(np.float64)))
    h0g = (np.minimum(sig, 0.5) + np.maximum(h_0, 0.0)).astype(np.float32)

    smalls = np.zeros((B, P, 16), dtype=np.float32)
    for hb in range(HB):
        blk = slice(hb * P, (hb + 1) * P)
        smalls[:, :, hb] = -bz[blk]
        smalls[:, :, 4 + hb] = bh[blk]
        smalls[:, :, 8 + hb] = h0g[:, blk]
        smalls[:, :, 12 + hb] = bz[blk]
    smalls = np.ascontiguousarray(smalls)

    in_maps = []
    for i in range(B):
        in_maps.append({
            "xT": xT[i],
            "wzT": wzT,
            "whT": whT,
            "smalls": smalls[i],
        })
    return in_maps


def kernel(x, h_0, Wz, bz, Wh, bh):
    global LAST_RESULT
    in_maps = _host_prep(x, h_0, Wz, bz, Wh, bh)
    nc = _build_nc()
    res = run_bass_kernel_spmd(
        nc,
        in_maps,
        core_ids=list(range(B)),
        trace=bool(int(os.environ.get("MINGRU_TRACE", "0"))),
    )
    LAST_RESULT = res
    out = np.empty((B, T, H), dtype=np.float32)
    for i in range(B):
        out[i] = np.asarray(res.results[i]["hT"]).astype(np.float32).T
    return out
